# revision 34
# baseline (speedup 1.0000x reference)
import sys

sys.path.insert(0, "/opt/trn_rl_repo")

import numpy as np
import ml_dtypes

import concourse.bacc as bacc
import concourse.bass as bass
import concourse.mybir as mybir
import concourse.tile as tile
from concourse.bass_utils import run_bass_kernel_spmd

F32 = mybir.dt.float32
BF16 = mybir.dt.bfloat16
AF = mybir.ActivationFunctionType
ALU = mybir.AluOpType
AX = mybir.AxisListType

# Problem constants (hardcoded per harness contract).
B, C, H, W = 4, 64, 128, 128
NT = 9          # 3x3 taps
NFF = 4         # factor*factor subpixels
NCORES = 8
HL = H // 2     # 64 coarse rows per core
NYB = 8         # y-blocks
YB = HL // NYB  # 8 rows per block
WR = YB + 2     # window rows per block (halo)
N = YB * 64     # elems per (tap, ff) product slab per partition

# units whose tap-sum runs as a DVE add-tree instead of PE matmuls
TREE_UNITS = {(b, 3) for b in (2, 3, 4, 5, 6)}

_cached = {}


def ap_of(t, off, dims):
    base = t[:]
    return bass.AP(base.tensor, base.offset + off, dims)


def build_nc():
    nc = bacc.Bacc("TRN2", target_bir_lowering=False, debug=False, num_devices=NCORES)

    hp2_d = nc.dram_tensor("hp2", [128, 66 * 130], BF16, kind="ExternalInput")
    hps_d = nc.dram_tensor("hps", [128, 66 * 130], BF16, kind="ExternalInput")
    # hT4: per-tap shifted planes, pairs (1,2),(3,4),(5,6),(7,8) interleaved
    # elementwise: [x, (k4, yy64, c64, p2)]; hS: tap-0 plane [x, (yy64, c64)]
    hT4_d = nc.dram_tensor("hT4", [128, 4 * 64 * 64 * 2], BF16, kind="ExternalInput")
    hS_d = nc.dram_tensor("hS", [128, 64 * 64], BF16, kind="ExternalInput")
    w1p_d = nc.dram_tensor("w1p", [128, 4 * 128], BF16, kind="ExternalInput")
    w1s_d = nc.dram_tensor("w1s", [64, 128], BF16, kind="ExternalInput")
    b1_d = nc.dram_tensor("b1c", [128, 1], F32, kind="ExternalInput")
    w2t_d = nc.dram_tensor("w2t", [128, 36], BF16, kind="ExternalInput")
    one_d = nc.dram_tensor("onec", [1, 128], BF16, kind="ExternalInput")
    b2r_d = nc.dram_tensor("b2r", [1, YB * 36], BF16, kind="ExternalInput")
    idb_d = nc.dram_tensor("idb", [128, 128], BF16, kind="ExternalInput")
    out_d = nc.dram_tensor("out", [64, H, 2 * W], BF16, kind="ExternalOutput")

    with tile.TileContext(nc) as tc:
        with (
            tc.tile_pool(name="const", bufs=1) as cpool,
            tc.tile_pool(name="ring", bufs=2) as ring,
            tc.tile_pool(name="mchunk", bufs=2) as mpool,
            tc.tile_pool(name="masks", bufs=2) as kpool,
            tc.tile_pool(name="prodp", bufs=3) as ppool,
            tc.tile_pool(name="accp", bufs=2) as apool,
            tc.tile_pool(name="orow", bufs=3) as opool,
            tc.tile_pool(name="ps1", bufs=2, space=bass.MemorySpace.PSUM) as pp1,
            tc.tile_pool(name="pst", bufs=1, space=bass.MemorySpace.PSUM) as ppt,
            tc.tile_pool(name="psa", bufs=3, space=bass.MemorySpace.PSUM) as ppa,
            tc.tile_pool(name="pso", bufs=2, space=bass.MemorySpace.PSUM) as ppo,
        ):
            # ---- constants ----
            w1p = cpool.tile([128, 4 * 128], BF16)
            w1s = cpool.tile([64, 128], BF16)
            b1 = cpool.tile([128, 1], F32)
            w2t = cpool.tile([128, 36], BF16)
            onec = cpool.tile([1, 128], BF16)
            b2r = cpool.tile([1, YB * 36], BF16)
            idb = cpool.tile([128, 128], BF16)
            nc.sync.dma_start(w1p[:], w1p_d[:])
            nc.sync.dma_start(w1s[:], w1s_d[:])
            nc.sync.dma_start(b1[:], b1_d[:])
            nc.sync.dma_start(w2t[:], w2t_d[:])
            nc.sync.dma_start(onec[:], one_d[:])
            nc.sync.dma_start(b2r[:], b2r_d[:])
            nc.sync.dma_start(idb[:], idb_d[:])

            # warm the ACT function tables before the pipeline needs them
            warm = cpool.tile([128, 1], F32)
            nc.vector.memset(warm[:], 0.0)
            nc.scalar.activation(warm[:], warm[:], AF.Exp)

            def dma_in(yb):
                r0 = yb * YB
                hp2b = ring.tile([128, WR * 130], BF16, tag="hp2b")
                hpsb = ring.tile([128, WR * 130], BF16, tag="hpsb")
                hT4b = ring.tile([128, 4 * YB * 128], BF16, tag="hT4b")
                hSb = ring.tile([128, YB * 64], BF16, tag="hSb")
                nc.sync.dma_start(hp2b[:], hp2_d[:, r0 * 130:(r0 + WR) * 130])
                nc.sync.dma_start(hpsb[:], hps_d[:, r0 * 130:(r0 + WR) * 130])
                nc.sync.dma_start(
                    hT4b[:],
                    ap_of(hT4_d, r0 * 128, [[4 * 64 * 128, 128], [64 * 128, 4], [1, YB * 128]]))
                nc.sync.dma_start(hSb[:], hS_d[:, r0 * 64:(r0 + YB) * 64])
                return hp2b, hpsb, hT4b, hSb

            def conv(yb, bufs):
                """conv1 -> relu, in 2 chunks of 4 rows."""
                hp2b, hpsb = bufs[0], bufs[1]
                m1 = mpool.tile([128, 2 * 512], BF16, tag="m1")
                for ic in range(2):
                    r = 4 * ic
                    ps1 = pp1.tile([128, 512], F32)
                    # 5 paired-tap matmuls: (t0,t1)x(buf, row off, col off)
                    plan = [(hp2b, 0, 0), (hpsb, 0, 2), (hp2b, 1, 1), (hp2b, 2, 0)]
                    for k, (buf, dr, dc) in enumerate(plan):
                        rhs = ap_of(buf, (r + dr) * 130 + dc,
                                    [[WR * 130, 128], [130, 4], [1, 128]])
                        nc.tensor.matmul(ps1[:], w1p[:, k * 128:(k + 1) * 128], rhs,
                                         start=(k == 0), stop=False)
                    rhs = ap_of(hp2b, (r + 2) * 130 + 2,
                                [[WR * 130, 64], [130, 4], [1, 128]])
                    nc.tensor.matmul(ps1[:], w1s[:], rhs, start=False, stop=True)
                    nc.scalar.activation(m1[:, ic * 512:(ic + 1) * 512], ps1[:],
                                         AF.Relu, bias=b1[:], scale=1.0)
                return m1

            def masks(yb, m1):
                """transposed conv2 (+bias) -> exp -> Z -> 1/Z -> dup masks."""
                pst = ppt.tile([128, YB * 36], F32)
                nc.tensor.matmul(pst[:], onec[:], b2r[:],
                                 start=True, stop=False, skip_group_check=True)
                for yl in range(YB):
                    nc.tensor.matmul(pst[:, yl * 36:(yl + 1) * 36],
                                     m1[:, yl * 128:(yl + 1) * 128], w2t[:],
                                     start=False, stop=(yl == YB - 1),
                                     skip_group_check=True)
                eT = kpool.tile([128, YB * 36], BF16, tag="eT")
                nc.scalar.activation(eT[:], pst[:], AF.Exp)

                zb = kpool.tile([128, YB * 4], F32, tag="zb")
                rz = kpool.tile([128, YB * 4], F32, tag="rz")
                z_in = ap_of(eT, 0, [[YB * 36, 128], [36, YB], [9, 4], [1, 9]])
                z_out = ap_of(zb, 0, [[YB * 4, 128], [4, YB], [1, 4]])
                nc.vector.tensor_reduce(z_out, z_in, AX.X, ALU.add)
                nc.vector.reciprocal(rz[:], zb[:])

                # nm slots per ff: [pad, t0, t1, ..., t8] (10 wide; pairs at
                # even offsets ff*10+2k+2 for 2x-mode alignment)
                nm = kpool.tile([128, YB * 40], BF16, tag="nm")
                for ff in range(NFF):
                    o_ap = ap_of(nm, ff * 10 + 1, [[YB * 40, 128], [40, YB], [1, 9]])
                    i0 = ap_of(eT, ff * 9, [[YB * 36, 128], [36, YB], [1, 9]])
                    i1 = ap_of(rz, ff, [[YB * 4, 128], [4, YB], [0, 9]])
                    nc.gpsimd.tensor_tensor(o_ap, i0, i1, ALU.mult)
                return nm

            def units(yb, state):
                """products (DVE pairs + Pool single) + tap-sum + evict."""
                hT4b, hSb, nm = state
                sA = apool.tile([128, NFF * N], BF16, tag="sA")
                for ff in range(NFF):
                    # prod layout: [t0 slab (N), pair k slabs (2N each)]
                    prod = ppool.tile([128, N + 4 * 2 * N], BF16, tag="prod")
                    # single tap t0 on Pool (1x mode, no dup needed)
                    i0 = ap_of(hSb, 0, [[YB * 64, 128], [1, N]])
                    i1 = ap_of(nm, ff * 10 + 1, [[YB * 40, 128], [40, YB], [0, 64]])
                    po = ap_of(prod, 0, [[NT * N, 128], [64, YB], [1, 64]])
                    nc.gpsimd.tensor_tensor(po, i0, i1, ALU.mult)
                    # 4 paired-tap products on DVE (2x mode)
                    for k in range(4):
                        i0 = ap_of(hT4b, k * (YB * 128),
                                   [[4 * YB * 128, 128], [128, YB], [2, 64], [1, 2]])
                        i1 = ap_of(nm, ff * 10 + 2 * k + 2,
                                   [[YB * 40, 128], [40, YB], [0, 64], [1, 2]])
                        po = ap_of(prod, N + k * 2 * N,
                                   [[NT * N, 128], [128, YB], [2, 64], [1, 2]])
                        nc.vector.tensor_tensor(po, i0, i1, ALU.mult)
                    fy, fx = ff // 2, ff % 2
                    if (yb, ff) in TREE_UNITS:
                        # DVE add-tree variant (rebalances PE -> DVE)
                        q1 = ppool.tile([128, 2 * N], BF16, tag="q1")
                        q2 = ppool.tile([128, 2 * N], BF16, tag="q2")
                        nc.vector.tensor_add(q1[:], prod[:, N:3 * N],
                                             prod[:, 3 * N:5 * N])
                        nc.vector.tensor_add(q2[:], prod[:, 5 * N:7 * N],
                                             prod[:, 7 * N:9 * N])
                        nc.vector.tensor_add(q1[:], q1[:], q2[:])
                        e_ap = ap_of(q2, 0, [[2 * N, 128], [1, N]])
                        nc.vector.tensor_tensor(
                            e_ap, ap_of(q1, 0, [[2 * N, 128], [2, N]]),
                            ap_of(q1, 1, [[2 * N, 128], [2, N]]), ALU.add)
                        o_ap = ap_of(sA, fx * 2 * N + fy,
                                     [[NFF * N, 128], [128, YB], [2, 64]])
                        nc.vector.tensor_tensor(
                            o_ap, e_ap, ap_of(prod, 0, [[NT * N, 128], [1, N]]),
                            ALU.add)
                    else:
                        psA = ppa.tile([128, N], F32)
                        nc.tensor.matmul(psA[:], idb[:], prod[:, 0:N],
                                         start=True, stop=False)
                        for k in range(4):
                            for p in range(2):
                                rhs = ap_of(prod, N + k * 2 * N + p,
                                            [[NT * N, 128], [2, N]])
                                nc.tensor.matmul(psA[:], idb[:], rhs,
                                                 start=False,
                                                 stop=(k == 3 and p == 1))
                        # evict interleaved: sA[x, (fx, y, (c,fy))]
                        o_ap = ap_of(sA, fx * 2 * N + fy,
                                     [[NFF * N, 128], [128, YB], [2, 64]])
                        i_ap = ap_of(psA, 0, [[N, 128], [64, YB], [1, 64]])
                        nc.scalar.copy(o_ap, i_ap)
                return sA

            def outT(yb, sA):
                """output transposes + interleave + DMA for block yb."""
                for g in range(YB // 2):
                    pso = ppo.tile([128, 512], BF16)
                    for yy in range(2):
                        yl = 2 * g + yy
                        for fx in range(2):
                            in_ap = ap_of(sA, fx * 2 * N + yl * 128,
                                          [[NFF * N, 128], [1, 128]])
                            o_ap = ap_of(pso, (yy * 2 + fx) * 128,
                                         [[512, 128], [1, 128]])
                            nc.tensor.transpose(o_ap, in_ap, idb[:])
                    orow = opool.tile([128, 512], BF16, tag="orow")
                    i_ap = ap_of(pso, 0, [[512, 128], [256, 2], [128, 2], [1, 128]])
                    o_ap = ap_of(orow, 0, [[512, 128], [256, 2], [1, 2], [2, 128]])
                    nc.scalar.copy(o_ap, i_ap)
                    y0 = yb * YB + 2 * g
                    for yy in range(2):
                        dst = ap_of(out_d, (2 * (y0 + yy)) * 256,
                                    [[H * 256, 64], [256, 2], [1, 256]])
                        nc.sync.dma_start(dst, orow[:, yy * 256:(yy + 1) * 256])

            # software pipeline, one block lag for the product/sum/output stage
            bufs = dma_in(0)
            m1 = conv(0, bufs)
            prev = (bufs[2], bufs[3], masks(0, m1))
            prev_sA = None
            for yb in range(1, NYB):
                bufs = dma_in(yb)
                m1 = conv(yb, bufs)
                sA = units(yb - 1, prev)
                nm = masks(yb, m1)
                if prev_sA is not None:
                    outT(yb - 2, prev_sA)
                prev = (bufs[2], bufs[3], nm)
                prev_sA = sA
            sA = units(NYB - 1, prev)
            outT(NYB - 2, prev_sA)
            outT(NYB - 1, sA)

    nc.compile()
    return nc


def prep_shared(W1, b1, W2, b2):
    W1 = np.asarray(W1, np.float32)
    b1 = np.asarray(b1, np.float32)
    W2 = np.asarray(W2, np.float32).reshape(36, 128)
    b2 = np.asarray(b2, np.float32)

    # paired-tap conv1 weights: pairs (t0 on parts 0:64, t1 on parts 64:128)
    # P1=(0,0)+(0,1) via hp2; P2=(0,2)+(1,0) via hps; P3=(1,1)+(1,2) via hp2;
    # P4=(2,0)+(2,1) via hp2; P5=(2,2) single via hp2 top half.
    pairs = [((0, 0), (0, 1)), ((0, 2), (1, 0)), ((1, 1), (1, 2)), ((2, 0), (2, 1))]
    w1p = np.zeros((128, 4 * 128), np.float32)
    for k, (ta, tb) in enumerate(pairs):
        w1p[0:64, k * 128:(k + 1) * 128] = W1[:, :, ta[0], ta[1]].T
        w1p[64:128, k * 128:(k + 1) * 128] = W1[:, :, tb[0], tb[1]].T
    w1s = np.ascontiguousarray(W1[:, :, 2, 2].T)

    # eb row k = (ff = k//9, t = k%9) <- original channel t*4+ff
    o_of_mp = np.array([t * 4 + ff for ff in range(4) for t in range(9)])
    w2t = np.ascontiguousarray((0.25 * W2[o_of_mp, :]).T)
    b2c = np.ascontiguousarray((0.25 * b2[o_of_mp]).reshape(36, 1))

    return {
        "w1p": w1p.astype(ml_dtypes.bfloat16),
        "w1s": w1s.astype(ml_dtypes.bfloat16),
        "b1c": b1.reshape(128, 1).astype(np.float32),
        "w2t": w2t.astype(ml_dtypes.bfloat16),
        "onec": np.ones((1, 128), dtype=ml_dtypes.bfloat16),
        "b2r": np.tile(b2c.ravel(), 8).reshape(1, -1).astype(ml_dtypes.bfloat16),
        "idb": np.eye(128, dtype=ml_dtypes.bfloat16),
    }


def kernel(h, W1, b1, W2, b2, _trace=False):
    h = np.asarray(h, np.float32)
    shared = prep_shared(W1, b1, W2, b2)

    hp = np.pad(h, ((0, 0), (0, 0), (1, 1), (1, 1)))  # [B, C, 130, 130]
    in_maps = []
    for core in range(NCORES):
        b, half = core // 2, core % 2
        y0 = half * HL
        win = hp[b, :, y0:y0 + 66, :]  # [64, 66, 130] f32
        winf = win.reshape(64, -1)
        hp2 = np.zeros((128, 66 * 130), np.float32)
        hp2[0:64] = winf
        hp2[64:128, 0:66 * 130 - 1] = winf[:, 1:]
        hps = np.zeros((128, 66 * 130), np.float32)
        hps[0:64] = winf
        hps[64:128, 0:66 * 130 - 128] = winf[:, 128:]
        w8 = 8.0 * win
        # per-tap shifted planes Q[t][c, yy, x] = 8*win[c, yy+dy, x+dx]
        Q = np.stack([w8[:, t // 3:t // 3 + 64, t % 3:t % 3 + 128]
                      for t in range(9)], axis=0)  # [9, 64c, 64y, 128x]
        # hT4[x, k, yy, c, p] = Q[1+2k+p][c, yy, x]
        hT4 = Q[1:9].reshape(4, 2, 64, 64, 128).transpose(4, 0, 3, 2, 1)
        hS = Q[0].transpose(2, 1, 0)  # [x, yy, c]
        m = dict(shared)
        m["hp2"] = hp2.astype(ml_dtypes.bfloat16)
        m["hps"] = hps.astype(ml_dtypes.bfloat16)
        m["hT4"] = np.ascontiguousarray(hT4).reshape(128, -1).astype(ml_dtypes.bfloat16)
        m["hS"] = np.ascontiguousarray(hS).reshape(128, -1).astype(ml_dtypes.bfloat16)
        in_maps.append(m)

    if "nc" not in _cached:
        _cached["nc"] = build_nc()
    res = run_bass_kernel_spmd(_cached["nc"], in_maps, core_ids=list(range(NCORES)),
                               trace=_trace)

    out = np.zeros((B, C, 2 * H, 2 * W), np.float32)
    for core in range(NCORES):
        b, half = core // 2, core % 2
        out[b, :, half * 128:(half + 1) * 128, :] = np.asarray(
            res.results[core]["out"], dtype=np.float32)
    if _trace:
        return out, res
    return out


# revision 41
# speedup vs baseline: 1.0240x; 1.0240x over previous
import sys

sys.path.insert(0, "/opt/trn_rl_repo")

import numpy as np
import ml_dtypes

import concourse.bacc as bacc
import concourse.bass as bass
import concourse.mybir as mybir
import concourse.tile as tile
from concourse.bass_utils import run_bass_kernel_spmd

F32 = mybir.dt.float32
BF16 = mybir.dt.bfloat16
AF = mybir.ActivationFunctionType
ALU = mybir.AluOpType
AX = mybir.AxisListType

# Problem constants (hardcoded per harness contract).
B, C, H, W = 4, 64, 128, 128
NT = 9          # 3x3 taps
NFF = 4         # factor*factor subpixels
NCORES = 8
HL = H // 2     # 64 coarse rows per core
NYB = 8         # y-blocks
YB = HL // NYB  # 8 rows per block
WR = YB + 2     # window rows per block (halo)
N = YB * 64     # elems per (tap, ff) product slab per partition

# units whose tap-sum runs as a DVE add-tree instead of PE matmuls
TREE_UNITS = {(2, 3), (4, 3), (6, 3)}

_cached = {}


def ap_of(t, off, dims):
    base = t[:]
    return bass.AP(base.tensor, base.offset + off, dims)


def build_nc():
    nc = bacc.Bacc("TRN2", target_bir_lowering=False, debug=False, num_devices=NCORES)

    hp2_d = nc.dram_tensor("hp2", [128, 66 * 130], BF16, kind="ExternalInput")
    hps_d = nc.dram_tensor("hps", [128, 66 * 130], BF16, kind="ExternalInput")
    hT3_d = nc.dram_tensor("hT3", [128, 3 * 66 * 64], BF16, kind="ExternalInput")
    w1p_d = nc.dram_tensor("w1p", [128, 4 * 128], BF16, kind="ExternalInput")
    w1s_d = nc.dram_tensor("w1s", [64, 128], BF16, kind="ExternalInput")
    b1_d = nc.dram_tensor("b1c", [128, 1], F32, kind="ExternalInput")
    w2t_d = nc.dram_tensor("w2t", [128, 36], BF16, kind="ExternalInput")
    one_d = nc.dram_tensor("onec", [1, 128], BF16, kind="ExternalInput")
    b2r_d = nc.dram_tensor("b2r", [1, YB * 36], BF16, kind="ExternalInput")
    idb_d = nc.dram_tensor("idb", [128, 128], BF16, kind="ExternalInput")
    out_d = nc.dram_tensor("out", [64, H, 2 * W], BF16, kind="ExternalOutput")

    with tile.TileContext(nc) as tc:
        with (
            tc.tile_pool(name="const", bufs=1) as cpool,
            tc.tile_pool(name="ring", bufs=2) as ring,
            tc.tile_pool(name="mchunk", bufs=2) as mpool,
            tc.tile_pool(name="masks", bufs=2) as kpool,
            tc.tile_pool(name="prodp", bufs=3) as ppool,
            tc.tile_pool(name="accp", bufs=2) as apool,
            tc.tile_pool(name="orow", bufs=3) as opool,
            tc.tile_pool(name="ps1", bufs=2, space=bass.MemorySpace.PSUM) as pp1,
            tc.tile_pool(name="pst", bufs=1, space=bass.MemorySpace.PSUM) as ppt,
            tc.tile_pool(name="psa", bufs=3, space=bass.MemorySpace.PSUM) as ppa,
            tc.tile_pool(name="pso", bufs=2, space=bass.MemorySpace.PSUM) as ppo,
        ):
            # ---- constants ----
            w1p = cpool.tile([128, 4 * 128], BF16)
            w1s = cpool.tile([64, 128], BF16)
            b1 = cpool.tile([128, 1], F32)
            w2t = cpool.tile([128, 36], BF16)
            onec = cpool.tile([1, 128], BF16)
            b2r = cpool.tile([1, YB * 36], BF16)
            idb = cpool.tile([128, 128], BF16)
            nc.sync.dma_start(w1p[:], w1p_d[:])
            nc.sync.dma_start(w1s[:], w1s_d[:])
            nc.sync.dma_start(b1[:], b1_d[:])

            def dma_in(yb):
                r0 = yb * YB
                hp2b = ring.tile([128, WR * 130], BF16, tag="hp2b")
                hpsb = ring.tile([128, WR * 130], BF16, tag="hpsb")
                hT3b = ring.tile([128, 3 * WR * 64], BF16, tag="hT3b")
                nc.sync.dma_start(hp2b[:], hp2_d[:, r0 * 130:(r0 + WR) * 130])
                nc.sync.dma_start(hpsb[:], hps_d[:, r0 * 130:(r0 + WR) * 130])
                nc.sync.dma_start(
                    hT3b[:],
                    ap_of(hT3_d, r0 * 64, [[3 * 66 * 64, 128], [66 * 64, 3], [1, WR * 64]]))
                return hp2b, hpsb, hT3b

            def conv(yb, bufs):
                """conv1 -> relu, in 2 chunks of 4 rows."""
                hp2b, hpsb, hT3b = bufs
                m1 = mpool.tile([128, 2 * 512], BF16, tag="m1")
                for ic in range(2):
                    r = 4 * ic
                    ps1 = pp1.tile([128, 512], F32)
                    # 5 paired-tap matmuls: (t0,t1)x(buf, row off, col off)
                    plan = [(hp2b, 0, 0), (hpsb, 0, 2), (hp2b, 1, 1), (hp2b, 2, 0)]
                    for k, (buf, dr, dc) in enumerate(plan):
                        rhs = ap_of(buf, (r + dr) * 130 + dc,
                                    [[WR * 130, 128], [130, 4], [1, 128]])
                        nc.tensor.matmul(ps1[:], w1p[:, k * 128:(k + 1) * 128], rhs,
                                         start=(k == 0), stop=False)
                    rhs = ap_of(hp2b, (r + 2) * 130 + 2,
                                [[WR * 130, 64], [130, 4], [1, 128]])
                    nc.tensor.matmul(ps1[:], w1s[:], rhs, start=False, stop=True)
                    nc.scalar.activation(m1[:, ic * 512:(ic + 1) * 512], ps1[:],
                                         AF.Relu, bias=b1[:], scale=1.0)
                return m1

            def masks(yb, m1):
                """transposed conv2 (+bias) -> exp -> Z -> 1/Z -> dup masks."""
                pst = ppt.tile([128, YB * 36], F32)
                nc.tensor.matmul(pst[:], onec[:], b2r[:],
                                 start=True, stop=False, skip_group_check=True)
                for yl in range(YB):
                    nc.tensor.matmul(pst[:, yl * 36:(yl + 1) * 36],
                                     m1[:, yl * 128:(yl + 1) * 128], w2t[:],
                                     start=False, stop=(yl == YB - 1),
                                     skip_group_check=True)
                eT = kpool.tile([128, YB * 36], BF16, tag="eT")
                nc.scalar.activation(eT[:], pst[:], AF.Exp)

                zb = kpool.tile([128, YB * 4], F32, tag="zb")
                rz = kpool.tile([128, YB * 4], F32, tag="rz")
                z_in = ap_of(eT, 0, [[YB * 36, 128], [36, YB], [9, 4], [1, 9]])
                z_out = ap_of(zb, 0, [[YB * 4, 128], [4, YB], [1, 4]])
                nc.vector.tensor_reduce(z_out, z_in, AX.X, ALU.add)
                nc.vector.reciprocal(rz[:], zb[:])

                nm = kpool.tile([128, YB * 72], BF16, tag="nm")
                for ff in range(NFF):
                    o_ap = ap_of(nm, ff * 18, [[YB * 72, 128], [72, YB], [2, 9], [1, 2]])
                    i0 = ap_of(eT, ff * 9, [[YB * 36, 128], [36, YB], [1, 9], [0, 2]])
                    i1 = ap_of(rz, ff, [[YB * 4, 128], [4, YB], [0, 9], [0, 2]])
                    nc.gpsimd.tensor_tensor(o_ap, i0, i1, ALU.mult)
                return nm

            def one_unit(yb, ff, state, sA):
                """products (Pool t0 + DVE t1-8) + tap-sum + evict, one unit."""
                hT3b, nm = state
                # tap 0 on the Pool engine, into its own tile (consumed last)
                prodS = ppool.tile([128, N], BF16, tag="prodS")
                i0 = ap_of(hT3b, 0, [[3 * WR * 64, 128], [64, YB], [1, 64]])
                i1 = ap_of(nm, ff * 18, [[YB * 72, 128], [72, YB], [0, 64]])
                po = ap_of(prodS, 0, [[N, 128], [64, YB], [1, 64]])
                nc.gpsimd.tensor_tensor(po, i0, i1, ALU.mult)
                # taps 1..8 on DVE (2x mode)
                prod = ppool.tile([128, 8 * N], BF16, tag="prod")
                for t in range(1, NT):
                    dy, dx = t // 3, t % 3
                    i0 = ap_of(hT3b, dx * (WR * 64) + dy * 64,
                               [[3 * WR * 64, 128], [64, YB], [2, 32], [1, 2]])
                    i1 = ap_of(nm, (ff * 9 + t) * 2,
                               [[YB * 72, 128], [72, YB], [0, 32], [1, 2]])
                    po = ap_of(prod, (t - 1) * N,
                               [[8 * N, 128], [64, YB], [2, 32], [1, 2]])
                    nc.vector.tensor_tensor(po, i0, i1, ALU.mult)
                fy, fx = ff // 2, ff % 2
                o_ap = ap_of(sA, fx * 2 * N + fy,
                             [[NFF * N, 128], [128, YB], [2, 64]])
                if (yb, ff) in TREE_UNITS:
                    # DVE add-tree variant (rebalances PE -> DVE)
                    tA = ppool.tile([128, 4 * N], BF16, tag="tA")
                    tB = ppool.tile([128, 2 * N], BF16, tag="tB")
                    nc.vector.tensor_add(tA[:], prod[:, 0:4 * N], prod[:, 4 * N:8 * N])
                    nc.vector.tensor_add(tB[:], tA[:, 0:2 * N], tA[:, 2 * N:4 * N])
                    nc.vector.tensor_add(tB[:, 0:N], tB[:, 0:N], tB[:, N:2 * N])
                    nc.vector.tensor_tensor(o_ap, tB[:, 0:N], prodS[:], ALU.add)
                else:
                    psA = ppa.tile([128, N], F32)
                    for t in range(8):
                        nc.tensor.matmul(psA[:], idb[:], prod[:, t * N:(t + 1) * N],
                                         start=(t == 0), stop=False)
                    nc.tensor.matmul(psA[:], idb[:], prodS[:],
                                     start=False, stop=True)
                    # evict interleaved: sA[x, (fx, y, (c,fy))]
                    i_ap = ap_of(psA, 0, [[N, 128], [64, YB], [1, 64]])
                    nc.scalar.copy(o_ap, i_ap)

            def outT_half(yb, sA, fx, orows):
                """output transposes + interleave for one fx half of a block."""
                for g in range(YB // 2):
                    pso = ppo.tile([128, 256], BF16)
                    for yy in range(2):
                        yl = 2 * g + yy
                        in_ap = ap_of(sA, fx * 2 * N + yl * 128,
                                      [[NFF * N, 128], [1, 128]])
                        o_ap = ap_of(pso, yy * 128, [[256, 128], [1, 128]])
                        nc.tensor.transpose(o_ap, in_ap, idb[:])
                    orow = orows[g]
                    i_ap = ap_of(pso, 0, [[256, 128], [128, 2], [1, 128]])
                    o_ap = ap_of(orow, fx, [[512, 128], [256, 2], [2, 128]])
                    nc.scalar.copy(o_ap, i_ap)

            def units(yb, state):
                """all four units of a block + its output stage."""
                sA = apool.tile([128, NFF * N], BF16, tag="sA")
                orows = [opool.tile([128, 512], BF16, tag=f"orow{g}",
                                    name=f"orow{g}")
                         for g in range(YB // 2)]
                for ff in (0, 2):
                    one_unit(yb, ff, state, sA)
                outT_half(yb, sA, 0, orows)
                for ff in (1, 3):
                    one_unit(yb, ff, state, sA)
                outT_half(yb, sA, 1, orows)
                for g in range(YB // 2):
                    y0 = yb * YB + 2 * g
                    for yy in range(2):
                        dst = ap_of(out_d, (2 * (y0 + yy)) * 256,
                                    [[H * 256, 64], [256, 2], [1, 256]])
                        nc.sync.dma_start(dst, orows[g][:, yy * 256:(yy + 1) * 256])

            # software pipeline, one block lag for the product/sum/output stage
            bufs = dma_in(0)
            # remaining consts after the first block's input slabs
            nc.sync.dma_start(w2t[:], w2t_d[:])
            nc.sync.dma_start(onec[:], one_d[:])
            nc.sync.dma_start(b2r[:], b2r_d[:])
            nc.sync.dma_start(idb[:], idb_d[:])
            # warm the ACT function tables before the pipeline needs them
            warm = cpool.tile([128, 1], F32)
            nc.vector.memset(warm[:], 0.0)
            nc.scalar.activation(warm[:], warm[:], AF.Exp)
            eb = conv(0, bufs)
            prev = (bufs[2], masks(0, eb))
            for yb in range(1, NYB):
                bufs = dma_in(yb)
                eb = conv(yb, bufs)
                units(yb - 1, prev)
                prev = (bufs[2], masks(yb, eb))
            units(NYB - 1, prev)

    nc.compile()
    return nc


def prep_shared(W1, b1, W2, b2):
    W1 = np.asarray(W1, np.float32)
    b1 = np.asarray(b1, np.float32)
    W2 = np.asarray(W2, np.float32).reshape(36, 128)
    b2 = np.asarray(b2, np.float32)

    # paired-tap conv1 weights: pairs (t0 on parts 0:64, t1 on parts 64:128)
    # P1=(0,0)+(0,1) via hp2; P2=(0,2)+(1,0) via hps; P3=(1,1)+(1,2) via hp2;
    # P4=(2,0)+(2,1) via hp2; P5=(2,2) single via hp2 top half.
    pairs = [((0, 0), (0, 1)), ((0, 2), (1, 0)), ((1, 1), (1, 2)), ((2, 0), (2, 1))]
    w1p = np.zeros((128, 4 * 128), np.float32)
    for k, (ta, tb) in enumerate(pairs):
        w1p[0:64, k * 128:(k + 1) * 128] = W1[:, :, ta[0], ta[1]].T
        w1p[64:128, k * 128:(k + 1) * 128] = W1[:, :, tb[0], tb[1]].T
    w1s = np.ascontiguousarray(W1[:, :, 2, 2].T)

    # eb row k = (ff = k//9, t = k%9) <- original channel t*4+ff
    o_of_mp = np.array([t * 4 + ff for ff in range(4) for t in range(9)])
    w2t = np.ascontiguousarray((0.25 * W2[o_of_mp, :]).T)
    b2c = np.ascontiguousarray((0.25 * b2[o_of_mp]).reshape(36, 1))

    return {
        "w1p": w1p.astype(ml_dtypes.bfloat16),
        "w1s": w1s.astype(ml_dtypes.bfloat16),
        "b1c": b1.reshape(128, 1).astype(np.float32),
        "w2t": w2t.astype(ml_dtypes.bfloat16),
        "onec": np.ones((1, 128), dtype=ml_dtypes.bfloat16),
        "b2r": np.tile(b2c.ravel(), 8).reshape(1, -1).astype(ml_dtypes.bfloat16),
        "idb": np.eye(128, dtype=ml_dtypes.bfloat16),
    }


def kernel(h, W1, b1, W2, b2, _trace=False):
    h = np.asarray(h, np.float32)
    shared = prep_shared(W1, b1, W2, b2)

    hp = np.pad(h, ((0, 0), (0, 0), (1, 1), (1, 1)))  # [B, C, 130, 130]
    in_maps = []
    for core in range(NCORES):
        b, half = core // 2, core % 2
        y0 = half * HL
        win = hp[b, :, y0:y0 + 66, :]  # [64, 66, 130] f32
        winf = win.reshape(64, -1)
        hp2 = np.zeros((128, 66 * 130), np.float32)
        hp2[0:64] = winf
        hp2[64:128, 0:66 * 130 - 1] = winf[:, 1:]
        hps = np.zeros((128, 66 * 130), np.float32)
        hps[0:64] = winf
        hps[64:128, 0:66 * 130 - 128] = winf[:, 128:]
        w8 = 8.0 * win
        # hT3[x, dx, yy, c] = 8*win[c, yy, x+dx]
        hT3 = np.stack([w8[:, :, d:d + 128] for d in range(3)], axis=0)  # [3,64,66,128]
        hT3 = np.ascontiguousarray(hT3.transpose(3, 0, 2, 1))  # [128,3,66,64]
        m = dict(shared)
        m["hp2"] = hp2.astype(ml_dtypes.bfloat16)
        m["hps"] = hps.astype(ml_dtypes.bfloat16)
        m["hT3"] = hT3.reshape(128, -1).astype(ml_dtypes.bfloat16)
        in_maps.append(m)

    if "nc" not in _cached:
        _cached["nc"] = build_nc()
    res = run_bass_kernel_spmd(_cached["nc"], in_maps, core_ids=list(range(NCORES)),
                               trace=_trace)

    out = np.zeros((B, C, 2 * H, 2 * W), np.float32)
    for core in range(NCORES):
        b, half = core // 2, core % 2
        out[b, :, half * 128:(half + 1) * 128, :] = np.asarray(
            res.results[core]["out"], dtype=np.float32)
    if _trace:
        return out, res
    return out


# revision 44
# speedup vs baseline: 1.1718x; 1.1443x over previous
import sys

sys.path.insert(0, "/opt/trn_rl_repo")

import numpy as np
import ml_dtypes

import concourse.bacc as bacc
import concourse.bass as bass
import concourse.mybir as mybir
import concourse.tile as tile
from concourse.bass_utils import run_bass_kernel_spmd

F32 = mybir.dt.float32
BF16 = mybir.dt.bfloat16
AF = mybir.ActivationFunctionType
ALU = mybir.AluOpType
AX = mybir.AxisListType

# Problem constants (hardcoded per harness contract).
B, C, H, W = 4, 64, 128, 128
NT = 9          # 3x3 taps
NFF = 4         # factor*factor subpixels
NCORES = 8
HL = H // 2     # 64 coarse rows per core
NYB = 8         # y-blocks
YB = HL // NYB  # 8 rows per block
WR = YB + 2     # window rows per block (halo)
N = YB * 64     # elems per (tap, ff) product slab per partition

_cached = {}


def ap_of(t, off, dims):
    base = t[:]
    return bass.AP(base.tensor, base.offset + off, dims)


def build_nc():
    nc = bacc.Bacc("TRN2", target_bir_lowering=False, debug=False, num_devices=NCORES)

    hp2_d = nc.dram_tensor("hp2", [128, 66 * 130], BF16, kind="ExternalInput")
    hps_d = nc.dram_tensor("hps", [128, 66 * 130], BF16, kind="ExternalInput")
    hT3_d = nc.dram_tensor("hT3", [128, 3 * 66 * 64], BF16, kind="ExternalInput")
    w1p_d = nc.dram_tensor("w1p", [128, 4 * 128], BF16, kind="ExternalInput")
    w1s_d = nc.dram_tensor("w1s", [64, 128], BF16, kind="ExternalInput")
    b1_d = nc.dram_tensor("b1c", [128, 1], F32, kind="ExternalInput")
    w2t_d = nc.dram_tensor("w2t", [128, 36], BF16, kind="ExternalInput")
    one_d = nc.dram_tensor("onec", [1, 128], BF16, kind="ExternalInput")
    b2r_d = nc.dram_tensor("b2r", [1, YB * 36], BF16, kind="ExternalInput")
    idb_d = nc.dram_tensor("idb", [128, 128], BF16, kind="ExternalInput")
    out_d = nc.dram_tensor("out", [64, H, 2 * W], BF16, kind="ExternalOutput")

    with tile.TileContext(nc) as tc:
        with (
            tc.tile_pool(name="const", bufs=1) as cpool,
            tc.tile_pool(name="ring", bufs=2) as ring,
            tc.tile_pool(name="mchunk", bufs=2) as mpool,
            tc.tile_pool(name="masks", bufs=2) as kpool,
            tc.tile_pool(name="prodp", bufs=3) as ppool,
            tc.tile_pool(name="accp", bufs=2) as apool,
            tc.tile_pool(name="orow", bufs=3) as opool,
            tc.tile_pool(name="ps1", bufs=2, space=bass.MemorySpace.PSUM) as pp1,
            tc.tile_pool(name="pst", bufs=1, space=bass.MemorySpace.PSUM) as ppt,
            tc.tile_pool(name="psa", bufs=3, space=bass.MemorySpace.PSUM) as ppa,
            tc.tile_pool(name="pso", bufs=2, space=bass.MemorySpace.PSUM) as ppo,
        ):
            # ---- constants ----
            w1p = cpool.tile([128, 4 * 128], BF16)
            w1s = cpool.tile([64, 128], BF16)
            b1 = cpool.tile([128, 1], F32)
            w2t = cpool.tile([128, 36], BF16)
            onec = cpool.tile([1, 128], BF16)
            b2r = cpool.tile([1, YB * 36], BF16)
            idb = cpool.tile([128, 128], BF16)
            nc.sync.dma_start(w1p[:], w1p_d[:])
            nc.sync.dma_start(w1s[:], w1s_d[:])
            nc.sync.dma_start(b1[:], b1_d[:])

            def late_consts():
                # needed only after conv1 of block 0; issued after its slabs
                nc.sync.dma_start(w2t[:], w2t_d[:])
                nc.sync.dma_start(onec[:], one_d[:])
                nc.sync.dma_start(b2r[:], b2r_d[:])
                nc.sync.dma_start(idb[:], idb_d[:])
                # warm the ACT function tables before the pipeline needs them
                warm = cpool.tile([128, 1], F32)
                nc.vector.memset(warm[:], 0.0)
                nc.scalar.activation(warm[:], warm[:], AF.Exp)

            def dma_in(yb):
                r0 = yb * YB
                hp2b = ring.tile([128, WR * 130], BF16, tag="hp2b")
                hpsb = ring.tile([128, WR * 130], BF16, tag="hpsb")
                hT3b = ring.tile([128, 3 * WR * 64], BF16, tag="hT3b")
                nc.sync.dma_start(hp2b[:], hp2_d[:, r0 * 130:(r0 + WR) * 130])
                nc.sync.dma_start(hpsb[:], hps_d[:, r0 * 130:(r0 + WR) * 130])
                nc.sync.dma_start(
                    hT3b[:],
                    ap_of(hT3_d, r0 * 64, [[3 * 66 * 64, 128], [66 * 64, 3], [1, WR * 64]]))
                return hp2b, hpsb, hT3b

            def conv(yb, bufs):
                """conv1 -> relu, in 2 chunks of 4 rows."""
                hp2b, hpsb, hT3b = bufs
                m1 = mpool.tile([128, 2 * 512], BF16, tag="m1")
                for ic in range(2):
                    r = 4 * ic
                    ps1 = pp1.tile([128, 512], F32)
                    # 5 paired-tap matmuls: (t0,t1)x(buf, row off, col off)
                    plan = [(hp2b, 0, 0), (hpsb, 0, 2), (hp2b, 1, 1), (hp2b, 2, 0)]
                    for k, (buf, dr, dc) in enumerate(plan):
                        rhs = ap_of(buf, (r + dr) * 130 + dc,
                                    [[WR * 130, 128], [130, 4], [1, 128]])
                        nc.tensor.matmul(ps1[:], w1p[:, k * 128:(k + 1) * 128], rhs,
                                         start=(k == 0), stop=False)
                    rhs = ap_of(hp2b, (r + 2) * 130 + 2,
                                [[WR * 130, 64], [130, 4], [1, 128]])
                    nc.tensor.matmul(ps1[:], w1s[:], rhs, start=False, stop=True)
                    nc.scalar.activation(m1[:, ic * 512:(ic + 1) * 512], ps1[:],
                                         AF.Relu, bias=b1[:], scale=1.0)
                return m1

            def masks(yb, m1):
                """transposed conv2 (+bias) -> exp -> Z -> 1/Z -> dup masks."""
                pst = ppt.tile([128, YB * 36], F32)
                nc.tensor.matmul(pst[:], onec[:], b2r[:],
                                 start=True, stop=False, skip_group_check=True)
                for yl in range(YB):
                    nc.tensor.matmul(pst[:, yl * 36:(yl + 1) * 36],
                                     m1[:, yl * 128:(yl + 1) * 128], w2t[:],
                                     start=False, stop=(yl == YB - 1),
                                     skip_group_check=True)
                eT = kpool.tile([128, YB * 36], BF16, tag="eT")
                nc.scalar.activation(eT[:], pst[:], AF.Exp)

                zb = kpool.tile([128, YB * 4], F32, tag="zb")
                rz = kpool.tile([128, YB * 4], F32, tag="rz")
                z_in = ap_of(eT, 0, [[YB * 36, 128], [36, YB], [9, 4], [1, 9]])
                z_out = ap_of(zb, 0, [[YB * 4, 128], [4, YB], [1, 4]])
                nc.vector.tensor_reduce(z_out, z_in, AX.X, ALU.add)
                nc.vector.reciprocal(rz[:], zb[:])

                nm = kpool.tile([128, YB * 72], BF16, tag="nm")
                for ff in range(NFF):
                    o_ap = ap_of(nm, ff * 18, [[YB * 72, 128], [72, YB], [2, 9], [1, 2]])
                    i0 = ap_of(eT, ff * 9, [[YB * 36, 128], [36, YB], [1, 9], [0, 2]])
                    i1 = ap_of(rz, ff, [[YB * 4, 128], [4, YB], [0, 9], [0, 2]])
                    nc.gpsimd.tensor_tensor(o_ap, i0, i1, ALU.mult)
                return nm

            def units(yb, state):
                """DVE products + PE tap-sum + evict for block yb."""
                hT3b, nm = state
                sA = apool.tile([128, NFF * N], BF16, tag="sA")
                for ff in range(NFF):
                    prod = ppool.tile([128, NT * N], BF16, tag="prod")
                    for dy in range(3):
                        for dx in range(3):
                            t = dy * 3 + dx
                            i0 = ap_of(hT3b, dx * (WR * 64) + dy * 64,
                                       [[3 * WR * 64, 128], [64, YB], [2, 32], [1, 2]])
                            i1 = ap_of(nm, (ff * 9 + t) * 2,
                                       [[YB * 72, 128], [72, YB], [0, 32], [1, 2]])
                            po = ap_of(prod, t * N,
                                       [[NT * N, 128], [64, YB], [2, 32], [1, 2]])
                            nc.vector.tensor_tensor(po, i0, i1, ALU.mult)
                    fy, fx = ff // 2, ff % 2
                    psA = ppa.tile([128, N], F32)
                    for t in range(NT):
                        nc.tensor.matmul(psA[:], idb[:], prod[:, t * N:(t + 1) * N],
                                         start=(t == 0), stop=(t == NT - 1))
                    # evict interleaved: sA[x, (fx, y, (c,fy))]
                    o_ap = ap_of(sA, fx * 2 * N + fy,
                                 [[NFF * N, 128], [128, YB], [2, 64]])
                    i_ap = ap_of(psA, 0, [[N, 128], [64, YB], [1, 64]])
                    nc.scalar.copy(o_ap, i_ap)
                return sA

            def outT(yb, sA):
                """output transposes + interleave + DMA for block yb."""
                for g in range(YB // 2):
                    pso = ppo.tile([128, 512], BF16)
                    for yy in range(2):
                        yl = 2 * g + yy
                        for fx in range(2):
                            in_ap = ap_of(sA, fx * 2 * N + yl * 128,
                                          [[NFF * N, 128], [1, 128]])
                            o_ap = ap_of(pso, (yy * 2 + fx) * 128,
                                         [[512, 128], [1, 128]])
                            nc.tensor.transpose(o_ap, in_ap, idb[:])
                    orow = opool.tile([128, 512], BF16, tag="orow")
                    i_ap = ap_of(pso, 0, [[512, 128], [256, 2], [128, 2], [1, 128]])
                    o_ap = ap_of(orow, 0, [[512, 128], [256, 2], [1, 2], [2, 128]])
                    nc.scalar.copy(o_ap, i_ap)
                    y0 = yb * YB + 2 * g
                    for yy in range(2):
                        dst = ap_of(out_d, (2 * (y0 + yy)) * 256,
                                    [[H * 256, 64], [256, 2], [1, 256]])
                        nc.sync.dma_start(dst, orow[:, yy * 256:(yy + 1) * 256])

            # software pipeline, one block lag for the product/sum/output stage
            bufs = dma_in(0)
            late_consts()
            eb = conv(0, bufs)
            prev = (bufs[2], masks(0, eb))
            prev_sA = None
            for yb in range(1, NYB):
                bufs = dma_in(yb)
                eb = conv(yb, bufs)
                sA = units(yb - 1, prev)
                nm = masks(yb, eb)
                if prev_sA is not None:
                    outT(yb - 2, prev_sA)
                prev = (bufs[2], nm)
                prev_sA = sA
            sA = units(NYB - 1, prev)
            outT(NYB - 2, prev_sA)
            outT(NYB - 1, sA)

    nc.compile()
    return nc


def prep_shared(W1, b1, W2, b2):
    W1 = np.asarray(W1, np.float32)
    b1 = np.asarray(b1, np.float32)
    W2 = np.asarray(W2, np.float32).reshape(36, 128)
    b2 = np.asarray(b2, np.float32)

    # paired-tap conv1 weights: pairs (t0 on parts 0:64, t1 on parts 64:128)
    # P1=(0,0)+(0,1) via hp2; P2=(0,2)+(1,0) via hps; P3=(1,1)+(1,2) via hp2;
    # P4=(2,0)+(2,1) via hp2; P5=(2,2) single via hp2 top half.
    pairs = [((0, 0), (0, 1)), ((0, 2), (1, 0)), ((1, 1), (1, 2)), ((2, 0), (2, 1))]
    w1p = np.zeros((128, 4 * 128), np.float32)
    for k, (ta, tb) in enumerate(pairs):
        w1p[0:64, k * 128:(k + 1) * 128] = W1[:, :, ta[0], ta[1]].T
        w1p[64:128, k * 128:(k + 1) * 128] = W1[:, :, tb[0], tb[1]].T
    w1s = np.ascontiguousarray(W1[:, :, 2, 2].T)

    # eb row k = (ff = k//9, t = k%9) <- original channel t*4+ff
    o_of_mp = np.array([t * 4 + ff for ff in range(4) for t in range(9)])
    w2t = np.ascontiguousarray((0.25 * W2[o_of_mp, :]).T)
    b2c = np.ascontiguousarray((0.25 * b2[o_of_mp]).reshape(36, 1))

    return {
        "w1p": w1p.astype(ml_dtypes.bfloat16),
        "w1s": w1s.astype(ml_dtypes.bfloat16),
        "b1c": b1.reshape(128, 1).astype(np.float32),
        "w2t": w2t.astype(ml_dtypes.bfloat16),
        "onec": np.ones((1, 128), dtype=ml_dtypes.bfloat16),
        "b2r": np.tile(b2c.ravel(), 8).reshape(1, -1).astype(ml_dtypes.bfloat16),
        "idb": np.eye(128, dtype=ml_dtypes.bfloat16),
    }


def kernel(h, W1, b1, W2, b2, _trace=False):
    h = np.asarray(h, np.float32)
    shared = prep_shared(W1, b1, W2, b2)

    hp = np.pad(h, ((0, 0), (0, 0), (1, 1), (1, 1)))  # [B, C, 130, 130]
    in_maps = []
    for core in range(NCORES):
        b, half = core // 2, core % 2
        y0 = half * HL
        win = hp[b, :, y0:y0 + 66, :]  # [64, 66, 130] f32
        winf = win.reshape(64, -1)
        hp2 = np.zeros((128, 66 * 130), np.float32)
        hp2[0:64] = winf
        hp2[64:128, 0:66 * 130 - 1] = winf[:, 1:]
        hps = np.zeros((128, 66 * 130), np.float32)
        hps[0:64] = winf
        hps[64:128, 0:66 * 130 - 128] = winf[:, 128:]
        w8 = 8.0 * win
        # hT3[x, dx, yy, c] = 8*win[c, yy, x+dx]
        hT3 = np.stack([w8[:, :, d:d + 128] for d in range(3)], axis=0)  # [3,64,66,128]
        hT3 = np.ascontiguousarray(hT3.transpose(3, 0, 2, 1))  # [128,3,66,64]
        m = dict(shared)
        m["hp2"] = hp2.astype(ml_dtypes.bfloat16)
        m["hps"] = hps.astype(ml_dtypes.bfloat16)
        m["hT3"] = hT3.reshape(128, -1).astype(ml_dtypes.bfloat16)
        in_maps.append(m)

    if "nc" not in _cached:
        _cached["nc"] = build_nc()
    res = run_bass_kernel_spmd(_cached["nc"], in_maps, core_ids=list(range(NCORES)),
                               trace=_trace)

    out = np.zeros((B, C, 2 * H, 2 * W), np.float32)
    for core in range(NCORES):
        b, half = core // 2, core % 2
        out[b, :, half * 128:(half + 1) * 128, :] = np.asarray(
            res.results[core]["out"], dtype=np.float32)
    if _trace:
        return out, res
    return out


# revision 46
# speedup vs baseline: 1.1879x; 1.0138x over previous
import sys

sys.path.insert(0, "/opt/trn_rl_repo")

import numpy as np
import ml_dtypes

import concourse.bacc as bacc
import concourse.bass as bass
import concourse.mybir as mybir
import concourse.tile as tile
from concourse.bass_utils import run_bass_kernel_spmd

F32 = mybir.dt.float32
BF16 = mybir.dt.bfloat16
AF = mybir.ActivationFunctionType
ALU = mybir.AluOpType
AX = mybir.AxisListType

# Problem constants (hardcoded per harness contract).
B, C, H, W = 4, 64, 128, 128
NT = 9          # 3x3 taps
NFF = 4         # factor*factor subpixels
NCORES = 8
HL = H // 2     # 64 coarse rows per core
NYB = 8         # y-blocks
YB = HL // NYB  # 8 rows per block
WR = YB + 2     # window rows per block (halo)
N = YB * 64     # elems per (tap, ff) product slab per partition

_cached = {}


def ap_of(t, off, dims):
    base = t[:]
    return bass.AP(base.tensor, base.offset + off, dims)


def build_nc():
    nc = bacc.Bacc("TRN2", target_bir_lowering=False, debug=False, num_devices=NCORES)

    hp2_d = nc.dram_tensor("hp2", [128, 66 * 130], BF16, kind="ExternalInput")
    hps_d = nc.dram_tensor("hps", [128, 66 * 130], BF16, kind="ExternalInput")
    hT3_d = nc.dram_tensor("hT3", [128, 3 * 66 * 64], BF16, kind="ExternalInput")
    w1p_d = nc.dram_tensor("w1p", [128, 4 * 128], BF16, kind="ExternalInput")
    w1s_d = nc.dram_tensor("w1s", [64, 128], BF16, kind="ExternalInput")
    b1_d = nc.dram_tensor("b1c", [128, 1], F32, kind="ExternalInput")
    w2t_d = nc.dram_tensor("w2t", [128, 36], BF16, kind="ExternalInput")
    one_d = nc.dram_tensor("onec", [1, 128], BF16, kind="ExternalInput")
    b2r_d = nc.dram_tensor("b2r", [1, YB * 36], BF16, kind="ExternalInput")
    idb_d = nc.dram_tensor("idb", [128, 128], BF16, kind="ExternalInput")
    out_d = nc.dram_tensor("out", [64, H, 2 * W], BF16, kind="ExternalOutput")

    with tile.TileContext(nc) as tc:
        with (
            tc.tile_pool(name="const", bufs=1) as cpool,
            tc.tile_pool(name="ring", bufs=2) as ring,
            tc.tile_pool(name="mchunk", bufs=2) as mpool,
            tc.tile_pool(name="masks", bufs=2) as kpool,
            tc.tile_pool(name="prodp", bufs=3) as ppool,
            tc.tile_pool(name="accp", bufs=2) as apool,
            tc.tile_pool(name="orow", bufs=3) as opool,
            tc.tile_pool(name="ps1", bufs=2, space=bass.MemorySpace.PSUM) as pp1,
            tc.tile_pool(name="pst", bufs=1, space=bass.MemorySpace.PSUM) as ppt,
            tc.tile_pool(name="psa", bufs=3, space=bass.MemorySpace.PSUM) as ppa,
            tc.tile_pool(name="pso", bufs=2, space=bass.MemorySpace.PSUM) as ppo,
        ):
            # ---- constants ----
            w1p = cpool.tile([128, 4 * 128], BF16)
            w1s = cpool.tile([64, 128], BF16)
            b1 = cpool.tile([128, 1], F32)
            w2t = cpool.tile([128, 36], BF16)
            onec = cpool.tile([1, 128], BF16)
            b2r = cpool.tile([1, YB * 36], BF16)
            idb = cpool.tile([128, 128], BF16)
            nc.sync.dma_start(w1p[:], w1p_d[:])
            nc.sync.dma_start(w1s[:], w1s_d[:])
            nc.sync.dma_start(b1[:], b1_d[:])

            def late_consts():
                # needed only after conv1 of block 0; issued after its slabs
                nc.sync.dma_start(w2t[:], w2t_d[:])
                nc.sync.dma_start(onec[:], one_d[:])
                nc.sync.dma_start(b2r[:], b2r_d[:])
                nc.sync.dma_start(idb[:], idb_d[:])
                # warm the ACT function tables before the pipeline needs them
                warm = cpool.tile([128, 1], F32)
                nc.vector.memset(warm[:], 0.0)
                nc.scalar.activation(warm[:], warm[:], AF.Exp)

            def dma_in(yb):
                r0 = yb * YB
                hp2b = ring.tile([128, WR * 130], BF16, tag="hp2b")
                hpsb = ring.tile([128, WR * 130], BF16, tag="hpsb")
                hT3b = ring.tile([128, 3 * WR * 64], BF16, tag="hT3b")
                nc.sync.dma_start(hp2b[:], hp2_d[:, r0 * 130:(r0 + WR) * 130])
                nc.sync.dma_start(hpsb[:], hps_d[:, r0 * 130:(r0 + WR) * 130])
                nc.sync.dma_start(
                    hT3b[:],
                    ap_of(hT3_d, r0 * 64, [[3 * 66 * 64, 128], [66 * 64, 3], [1, WR * 64]]))
                return hp2b, hpsb, hT3b

            def conv(yb, bufs):
                """conv1 -> relu, in 2 chunks of 4 rows."""
                hp2b, hpsb, hT3b = bufs
                m1 = mpool.tile([128, 2 * 512], BF16, tag="m1")
                for ic in range(2):
                    r = 4 * ic
                    ps1 = pp1.tile([128, 512], F32)
                    # 5 paired-tap matmuls: (t0,t1)x(buf, row off, col off)
                    plan = [(hp2b, 0, 0), (hpsb, 0, 2), (hp2b, 1, 1), (hp2b, 2, 0)]
                    for k, (buf, dr, dc) in enumerate(plan):
                        rhs = ap_of(buf, (r + dr) * 130 + dc,
                                    [[WR * 130, 128], [130, 4], [1, 128]])
                        nc.tensor.matmul(ps1[:], w1p[:, k * 128:(k + 1) * 128], rhs,
                                         start=(k == 0), stop=False)
                    rhs = ap_of(hp2b, (r + 2) * 130 + 2,
                                [[WR * 130, 64], [130, 4], [1, 128]])
                    nc.tensor.matmul(ps1[:], w1s[:], rhs, start=False, stop=True)
                    nc.scalar.activation(m1[:, ic * 512:(ic + 1) * 512], ps1[:],
                                         AF.Relu, bias=b1[:], scale=1.0)
                return m1

            def masks(yb, m1):
                """transposed conv2 (+bias) -> exp -> Z -> 1/Z -> dup masks."""
                pst = ppt.tile([128, YB * 36], F32)
                nc.tensor.matmul(pst[:], onec[:], b2r[:],
                                 start=True, stop=False, skip_group_check=True)
                for yl in range(YB):
                    nc.tensor.matmul(pst[:, yl * 36:(yl + 1) * 36],
                                     m1[:, yl * 128:(yl + 1) * 128], w2t[:],
                                     start=False, stop=(yl == YB - 1),
                                     skip_group_check=True)
                eT = kpool.tile([128, YB * 36], BF16, tag="eT")
                nc.scalar.activation(eT[:], pst[:], AF.Exp)

                zb = kpool.tile([128, YB * 4], F32, tag="zb")
                rz = kpool.tile([128, YB * 4], F32, tag="rz")
                z_in = ap_of(eT, 0, [[YB * 36, 128], [36, YB], [9, 4], [1, 9]])
                z_out = ap_of(zb, 0, [[YB * 4, 128], [4, YB], [1, 4]])
                nc.vector.tensor_reduce(z_out, z_in, AX.X, ALU.add)
                nc.vector.reciprocal(rz[:], zb[:])

                nm = kpool.tile([128, YB * 72], BF16, tag="nm")
                for ff in range(NFF):
                    o_ap = ap_of(nm, ff * 18, [[YB * 72, 128], [72, YB], [2, 9], [1, 2]])
                    i0 = ap_of(eT, ff * 9, [[YB * 36, 128], [36, YB], [1, 9], [0, 2]])
                    i1 = ap_of(rz, ff, [[YB * 4, 128], [4, YB], [0, 9], [0, 2]])
                    nc.gpsimd.tensor_tensor(o_ap, i0, i1, ALU.mult)
                return nm

            def one_unit(yb, ff, state, sA):
                """DVE products + PE tap-sum + evict for one (block, ff)."""
                hT3b, nm = state
                prod = ppool.tile([128, NT * N], BF16, tag="prod")
                for dy in range(3):
                    for dx in range(3):
                        t = dy * 3 + dx
                        i0 = ap_of(hT3b, dx * (WR * 64) + dy * 64,
                                   [[3 * WR * 64, 128], [64, YB], [2, 32], [1, 2]])
                        i1 = ap_of(nm, (ff * 9 + t) * 2,
                                   [[YB * 72, 128], [72, YB], [0, 32], [1, 2]])
                        po = ap_of(prod, t * N,
                                   [[NT * N, 128], [64, YB], [2, 32], [1, 2]])
                        nc.vector.tensor_tensor(po, i0, i1, ALU.mult)
                fy, fx = ff // 2, ff % 2
                psA = ppa.tile([128, N], F32)
                for t in range(NT):
                    nc.tensor.matmul(psA[:], idb[:], prod[:, t * N:(t + 1) * N],
                                     start=(t == 0), stop=(t == NT - 1))
                # evict interleaved: sA[x, (fx, y, (c,fy))]
                o_ap = ap_of(sA, fx * 2 * N + fy,
                             [[NFF * N, 128], [128, YB], [2, 64]])
                i_ap = ap_of(psA, 0, [[N, 128], [64, YB], [1, 64]])
                nc.scalar.copy(o_ap, i_ap)

            def outT_half(sA, fx, orows):
                """output transposes + interleave for one fx half of a block."""
                for g in range(YB // 2):
                    pso = ppo.tile([128, 256], BF16)
                    for yy in range(2):
                        yl = 2 * g + yy
                        in_ap = ap_of(sA, fx * 2 * N + yl * 128,
                                      [[NFF * N, 128], [1, 128]])
                        o_ap = ap_of(pso, yy * 128, [[256, 128], [1, 128]])
                        nc.tensor.transpose(o_ap, in_ap, idb[:])
                    i_ap = ap_of(pso, 0, [[256, 128], [128, 2], [1, 128]])
                    o_ap = ap_of(orows[g], fx, [[512, 128], [256, 2], [2, 128]])
                    nc.scalar.copy(o_ap, i_ap)

            def units(yb, state):
                """all four units of a block + its output stage, fx-pipelined."""
                sA = apool.tile([128, NFF * N], BF16, tag="sA")
                orows = [opool.tile([128, 512], BF16, tag=f"orow{g}",
                                    name=f"orow{g}")
                         for g in range(YB // 2)]
                one_unit(yb, 0, state, sA)
                one_unit(yb, 2, state, sA)
                outT_half(sA, 0, orows)
                one_unit(yb, 1, state, sA)
                one_unit(yb, 3, state, sA)
                outT_half(sA, 1, orows)
                for g in range(YB // 2):
                    y0 = yb * YB + 2 * g
                    for yy in range(2):
                        dst = ap_of(out_d, (2 * (y0 + yy)) * 256,
                                    [[H * 256, 64], [256, 2], [1, 256]])
                        nc.sync.dma_start(dst, orows[g][:, yy * 256:(yy + 1) * 256])

            # software pipeline, one block lag for the product/sum/output stage
            bufs = dma_in(0)
            late_consts()
            eb = conv(0, bufs)
            prev = (bufs[2], masks(0, eb))
            for yb in range(1, NYB):
                bufs = dma_in(yb)
                eb = conv(yb, bufs)
                units(yb - 1, prev)
                prev = (bufs[2], masks(yb, eb))
            units(NYB - 1, prev)

    nc.compile()
    return nc


def prep_shared(W1, b1, W2, b2):
    W1 = np.asarray(W1, np.float32)
    b1 = np.asarray(b1, np.float32)
    W2 = np.asarray(W2, np.float32).reshape(36, 128)
    b2 = np.asarray(b2, np.float32)

    # paired-tap conv1 weights: pairs (t0 on parts 0:64, t1 on parts 64:128)
    # P1=(0,0)+(0,1) via hp2; P2=(0,2)+(1,0) via hps; P3=(1,1)+(1,2) via hp2;
    # P4=(2,0)+(2,1) via hp2; P5=(2,2) single via hp2 top half.
    pairs = [((0, 0), (0, 1)), ((0, 2), (1, 0)), ((1, 1), (1, 2)), ((2, 0), (2, 1))]
    w1p = np.zeros((128, 4 * 128), np.float32)
    for k, (ta, tb) in enumerate(pairs):
        w1p[0:64, k * 128:(k + 1) * 128] = W1[:, :, ta[0], ta[1]].T
        w1p[64:128, k * 128:(k + 1) * 128] = W1[:, :, tb[0], tb[1]].T
    w1s = np.ascontiguousarray(W1[:, :, 2, 2].T)

    # eb row k = (ff = k//9, t = k%9) <- original channel t*4+ff
    o_of_mp = np.array([t * 4 + ff for ff in range(4) for t in range(9)])
    w2t = np.ascontiguousarray((0.25 * W2[o_of_mp, :]).T)
    b2c = np.ascontiguousarray((0.25 * b2[o_of_mp]).reshape(36, 1))

    return {
        "w1p": w1p.astype(ml_dtypes.bfloat16),
        "w1s": w1s.astype(ml_dtypes.bfloat16),
        "b1c": b1.reshape(128, 1).astype(np.float32),
        "w2t": w2t.astype(ml_dtypes.bfloat16),
        "onec": np.ones((1, 128), dtype=ml_dtypes.bfloat16),
        "b2r": np.tile(b2c.ravel(), 8).reshape(1, -1).astype(ml_dtypes.bfloat16),
        "idb": np.eye(128, dtype=ml_dtypes.bfloat16),
    }


def kernel(h, W1, b1, W2, b2, _trace=False):
    h = np.asarray(h, np.float32)
    shared = prep_shared(W1, b1, W2, b2)

    hp = np.pad(h, ((0, 0), (0, 0), (1, 1), (1, 1)))  # [B, C, 130, 130]
    in_maps = []
    for core in range(NCORES):
        b, half = core // 2, core % 2
        y0 = half * HL
        win = hp[b, :, y0:y0 + 66, :]  # [64, 66, 130] f32
        winf = win.reshape(64, -1)
        hp2 = np.zeros((128, 66 * 130), np.float32)
        hp2[0:64] = winf
        hp2[64:128, 0:66 * 130 - 1] = winf[:, 1:]
        hps = np.zeros((128, 66 * 130), np.float32)
        hps[0:64] = winf
        hps[64:128, 0:66 * 130 - 128] = winf[:, 128:]
        w8 = 8.0 * win
        # hT3[x, dx, yy, c] = 8*win[c, yy, x+dx]
        hT3 = np.stack([w8[:, :, d:d + 128] for d in range(3)], axis=0)  # [3,64,66,128]
        hT3 = np.ascontiguousarray(hT3.transpose(3, 0, 2, 1))  # [128,3,66,64]
        m = dict(shared)
        m["hp2"] = hp2.astype(ml_dtypes.bfloat16)
        m["hps"] = hps.astype(ml_dtypes.bfloat16)
        m["hT3"] = hT3.reshape(128, -1).astype(ml_dtypes.bfloat16)
        in_maps.append(m)

    if "nc" not in _cached:
        _cached["nc"] = build_nc()
    res = run_bass_kernel_spmd(_cached["nc"], in_maps, core_ids=list(range(NCORES)),
                               trace=_trace)

    out = np.zeros((B, C, 2 * H, 2 * W), np.float32)
    for core in range(NCORES):
        b, half = core // 2, core % 2
        out[b, :, half * 128:(half + 1) * 128, :] = np.asarray(
            res.results[core]["out"], dtype=np.float32)
    if _trace:
        return out, res
    return out
